# revision 31
# baseline (speedup 1.0000x reference)
"""Trainium2 Bass kernel for nn_KAN_63230508532179 (dense_mlp).

Model (per reference):
  h = gelu(x[:,:,None] * bw1 + bb1)            # [B,1000,16]
  f = tanh(einsum('bnh,noh->bno', h, bw2)+bb2) # [B,1000,8]
  z = f.reshape(B, 8000)
  z = gelu(z @ wc1.T + bc1)                    # [B,256]
  z = gelu(z @ wc2.T + bc2)                    # [B,128]
  y = z @ wc3.T + bc3                          # [B,300]

The graded time is dominated by host->device input streaming
(~5.9 GB/s shared across cores), so the design minimizes total wire
bytes.  Strategy: BRANCH-parallel across the 8 cores — core c owns 126
of the 1000 branches over the FULL batch, so the combiner weight wc1
(the largest tensor) is split 8 ways instead of replicated, and the
branch layer-1 weight is never inflated: a per-group 0/1 selector
matrix (built on device with affine_select) broadcasts x rows into the
(j,k) layout via a K=128 matmul, and the actual bw1/bb1 are applied as
per-partition scale/bias in a fused DVE tensor_scalar.  Each core
accumulates a partial z1 = f @ wc1_c.T over its branches for all 4096
batch rows; an on-device ReduceScatter sums partials and hands core c
the batch slice [512c, 512c+512), on which it runs the small combiner
tail.

Wire formats: x ships as int8 with a runtime global scale that is
folded into the DVE prescale's scale vector; the wc1 K-slice ships as
int8 with runtime per-output-row scales that are folded into the tail
gelu's per-partition scale AP (both upcast to bf16 on device, zero
extra scaling ops); branch layer-2 weights ship semi-compact
[128, 128] and are expanded to the block-diagonal matmul form on
device; remaining tensors are bf16; output is fp16.  Validated rel
err ~0.016 vs the 2e-2 budget.  Per-core wire bytes ~0.94 MB (vs
12.9 MB for the batch-parallel baseline with replicated weights).
"""

import os
import sys
from contextlib import ExitStack

sys.path.insert(0, "/opt/trn_rl_repo")
os.environ.setdefault("MYCRO_LOCAL_CACHE", "1")

import numpy as np
import ml_dtypes

import concourse.bass as bass
import concourse.tile as tile
from concourse import bacc, mybir
from concourse.bass_utils import run_bass_kernel_spmd

BF16 = mybir.dt.bfloat16
F32 = mybir.dt.float32
F16 = mybir.dt.float16
I8 = mybir.dt.int8
NPBF16 = ml_dtypes.bfloat16

B, N, H1, H2 = 4096, 1000, 16, 8
C1, C2, OUT = 256, 128, 300
NCORES = 8
NBR = 126                 # real branch slots per core (last core: 118)
NBP = 128                 # padded branch slots per core
NG = 16                   # groups of 8 branches per core
NPAIR = 8                 # group pairs (= wc1 K-chunks of 128)
BCH = 8                   # batch chunks
BC = 512                  # batch per chunk (= per-core tail batch)

_CACHE = {}

# (name, dtype, cols) of every input tensor, in blob packing order:
# f32 block first, then bf16, then int8 — keeps row offsets aligned.
BLOB_ORDER = [
    ("sc", "f32", NG), ("sb", "f32", NG), ("b2", "f32", NPAIR),
    ("swc", "f32", 2), ("bc1", "f32", 2), ("bc2", "f32", 1),
    ("bc3", "f32", 3),
    ("w2", "bf16", NG * 8), ("wc2", "bf16", 256), ("wc3", "bf16", OUT),
    ("xt", "i8", B), ("wc1", "i8", NPAIR * C1),
]
_DT_SIZE = {"f32": 4, "bf16": 2, "i8": 1}
BLOB_OFF = {}
_o = 0
for _n, _t, _c in BLOB_ORDER:
    BLOB_OFF[_n] = _o
    _o += _DT_SIZE[_t] * _c
BLOB_BYTES = _o                   # 7704 bytes per partition row


def _build_program():
    if "nc" in _CACHE:
        return _CACHE["nc"]

    nc = bacc.Bacc("TRN2", target_bir_lowering=False, debug=False,
                   num_devices=NCORES)

    # All inputs ship as ONE packed uint8 tensor per core (identical
    # bytes; avoids any per-tensor streaming overhead in the harness).
    # Layout per partition row, in BLOB_ORDER: f32 block, bf16 block,
    # int8 block — every tensor's row-offset stays dtype-aligned.
    blob_d = nc.dram_tensor("blob", [128, BLOB_BYTES], mybir.dt.uint8,
                            kind="ExternalInput")
    out_d = nc.dram_tensor("out", [OUT, BC], F16, kind="ExternalOutput")

    AF = mybir.ActivationFunctionType
    ALU = mybir.AluOpType

    with ExitStack() as ctx:
        tc = ctx.enter_context(tile.TileContext(nc))
        consts = ctx.enter_context(tc.tile_pool(name="consts", bufs=1))
        sel_p = ctx.enter_context(tc.tile_pool(name="selp", bufs=2))
        h_pool = ctx.enter_context(tc.tile_pool(name="h", bufs=3))
        g_pool = ctx.enter_context(tc.tile_pool(name="g", bufs=3))
        f_pool = ctx.enter_context(tc.tile_pool(name="f", bufs=3))
        z_pool = ctx.enter_context(tc.tile_pool(name="z", bufs=2))
        ps_h = ctx.enter_context(tc.tile_pool(name="psh", bufs=2, space="PSUM"))
        ps_f = ctx.enter_context(tc.tile_pool(name="psf", bufs=2, space="PSUM"))
        ps_z = ctx.enter_context(tc.tile_pool(name="psz", bufs=2, space="PSUM"))
        dram = ctx.enter_context(tc.tile_pool(name="dram", bufs=1, space="DRAM"))

        # load each tensor from its byte-range of the packed blob
        def load(name, dt, tag, eng=None):
            cols = dict((n, c) for n, _, c in BLOB_ORDER)[name]
            nb = cols * mybir.dt.size(dt)
            off = BLOB_OFF[name]
            s = consts.tile([128, cols], dt, tag=tag)
            (eng or nc.scalar).dma_start(
                out=s[:], in_=blob_d[:, off:off + nb].bitcast(dt))
            return s

        xt8_sb = load("xt", I8, "xt8", nc.sync)
        sc_sb = load("sc", F32, "sc")
        sbb_sb = load("sb", F32, "sb")
        w2c_sb = load("w2", BF16, "w2c")
        b2_sb = load("b2", F32, "b2")
        wc1i_sb = load("wc1", I8, "wc1i", nc.sync)
        swc_sb = load("swc", F32, "swc")
        wc2_sb = load("wc2", BF16, "wc2")
        bc1_sb = load("bc1", F32, "bc1")
        bc2_sb = load("bc2", F32, "bc2")
        wc3_sb = load("wc3", BF16, "wc3")
        bc3_sb = load("bc3", F32, "bc3")

        # upcast x and wc1 from their int8 wire format to bf16 for the
        # matmuls (scales are folded into sc and the tail gelu's scale AP)
        xt_sb = consts.tile([128, B], BF16, tag="xt")
        for hb in range(2):
            nc.vector.tensor_copy(xt_sb[:, 2048 * hb:2048 * (hb + 1)],
                                  xt8_sb[:, 2048 * hb:2048 * (hb + 1)])
        wc1_sb = consts.tile([128, NPAIR * C1], BF16, tag="wc1")
        nc.vector.tensor_copy(wc1_sb[:], wc1i_sb[:])

        # expand semi-compact w2c [16j+k, 8g+o] into the block-diagonal
        # W2 [16j+k, 64g + 8j + o] (zeros elsewhere)
        w2_sb = consts.tile([128, NG * 64], BF16, tag="w2")
        nc.gpsimd.memset(w2_sb[:], 0.0)
        for j in range(8):
            src = w2c_sb[16 * j:16 * (j + 1), :].rearrange(
                "p (g o) -> p g o", g=NG)
            dst = w2_sb[16 * j:16 * (j + 1), :].rearrange(
                "p (g q) -> p g q", g=NG)[:, :, 8 * j:8 * (j + 1)]
            nc.scalar.dma_start(out=dst, in_=src)

        # Selector matrices S_u[p, m] = 1 iff 0 <= m - 16*(p - 8u) < 16,
        # i.e. matmul(S_u.T @ x) broadcasts x row 8u+j to h rows 16j..16j+15.
        sel = consts.tile([128, NG * 128], BF16, tag="sel")
        ones = consts.tile([128, 128], BF16, tag="ones")
        nc.gpsimd.memset(ones[:], 1.0)
        for u in range(NG):
            tmp = sel_p.tile([128, 128], BF16)
            # keep where m - 16p + 128u >= 0
            nc.gpsimd.affine_select(
                out=tmp[:], in_=ones[:], pattern=[[1, 128]],
                compare_op=ALU.is_ge, fill=0.0,
                base=128 * u, channel_multiplier=-16)
            # keep where 15 + 16p - 128u - m >= 0
            nc.gpsimd.affine_select(
                out=sel[:, 128 * u:128 * (u + 1)], in_=tmp[:],
                pattern=[[-1, 128]], compare_op=ALU.is_ge, fill=0.0,
                base=15 - 128 * u, channel_multiplier=16)

        z1_part = dram.tile([BCH * C1, BC], F32, tag="z1p")   # [2048, 512]
        z1_my = dram.tile([C1, BC], F32, tag="z1m")

        # ---- main loop: batch chunks x group pairs ----
        for nb in range(BCH):
            z_ps = ps_z.tile([128, 1024], F32)
            for t in range(NPAIR):
                hg = h_pool.tile([128, 1024], BF16)
                for half in range(2):
                    g = 2 * t + half
                    h_ps = ps_h.tile([128, 512], F32)
                    nc.tensor.matmul(
                        h_ps[:], lhsT=sel[:, 128 * g:128 * (g + 1)],
                        rhs=xt_sb[:, BC * nb:BC * (nb + 1)],
                        start=True, stop=True)
                    nc.vector.tensor_scalar(
                        out=hg[:, 512 * half:512 * (half + 1)], in0=h_ps[:],
                        scalar1=sc_sb[:, g:g + 1], scalar2=sbb_sb[:, g:g + 1],
                        op0=ALU.mult, op1=ALU.add)
                hG = g_pool.tile([128, 1024], BF16)
                nc.scalar.activation(hG[:], hg[:], AF.Gelu)
                f_ps = ps_f.tile([128, 512], F32)
                for half in range(2):
                    g = 2 * t + half
                    nc.tensor.matmul(
                        f_ps[64 * half:64 * (half + 1), :],
                        lhsT=w2_sb[:, 64 * g:64 * (g + 1)],
                        rhs=hG[:, 512 * half:512 * (half + 1)],
                        start=True, stop=True)
                fT = f_pool.tile([128, 512], BF16)
                nc.scalar.activation(fT[:], f_ps[:], AF.Tanh,
                                     bias=b2_sb[:, t:t + 1], scale=1.0)
                last = t == NPAIR - 1
                nc.tensor.matmul(z_ps[:, 0:512],
                                 lhsT=wc1_sb[:, 256 * t:256 * t + 128],
                                 rhs=fT[:], start=(t == 0), stop=last,
                                 skip_group_check=True)
                nc.tensor.matmul(z_ps[:, 512:1024],
                                 lhsT=wc1_sb[:, 256 * t + 128:256 * t + 256],
                                 rhs=fT[:], start=(t == 0), stop=last,
                                 skip_group_check=True)
            z_sb = z_pool.tile([128, 1024], F32, tag="z_sb")
            nc.vector.tensor_copy(z_sb[:], z_ps[:])
            nc.sync.dma_start(out=z1_part[256 * nb:256 * nb + 128, :],
                              in_=z_sb[:, 0:512])
            nc.sync.dma_start(out=z1_part[256 * nb + 128:256 * nb + 256, :],
                              in_=z_sb[:, 512:1024])

        # ---- cross-core reduction: core c gets batch chunk c, summed ----
        nc.gpsimd.collective_compute(
            "ReduceScatter", ALU.add,
            replica_groups=[list(range(NCORES))],
            ins=[z1_part.opt()], outs=[z1_my.opt()],
        )

        # ---- combiner tail on this core's 512-row batch slice ----
        z1g = []
        for hk in range(2):
            z1f = z_pool.tile([128, 512], F32, tag="z1f")
            nc.sync.dma_start(out=z1f[:],
                              in_=z1_my[128 * hk:128 * (hk + 1), :])
            zg = z_pool.tile([128, 512], BF16, tag="z1g")
            nc.scalar.activation(zg[:], z1f[:], AF.Gelu,
                                 bias=bc1_sb[:, hk:hk + 1],
                                 scale=swc_sb[:, hk:hk + 1])
            z1g.append(zg)

        z2_ps = ps_h.tile([128, 512], F32, tag="h_ps")
        nc.tensor.matmul(z2_ps[:], lhsT=wc2_sb[:, 0:128], rhs=z1g[0][:],
                         start=True, stop=False, skip_group_check=True)
        nc.tensor.matmul(z2_ps[:], lhsT=wc2_sb[:, 128:256], rhs=z1g[1][:],
                         start=False, stop=True, skip_group_check=True)
        z2 = z_pool.tile([128, 512], BF16, tag="z2")
        nc.scalar.activation(z2[:], z2_ps[:], AF.Gelu,
                             bias=bc2_sb[:, 0:1], scale=1.0)

        for i, mrows in ((0, 128), (1, 128), (2, 44)):
            o_ps = ps_f.tile([128, 512], F32, tag="f_ps")
            nc.tensor.matmul(o_ps[0:mrows, :],
                             lhsT=wc3_sb[:, 128 * i:128 * i + mrows],
                             rhs=z2[:], start=True, stop=True)
            o_sb = z_pool.tile([128, 512], F16, tag="o")
            nc.vector.tensor_scalar_add(o_sb[0:mrows, :], o_ps[0:mrows, :],
                                        bc3_sb[0:mrows, i:i + 1])
            nc.sync.dma_start(out=out_d[128 * i:128 * i + mrows, :],
                              in_=o_sb[0:mrows, :])

    nc.compile()
    _CACHE["nc"] = nc
    return nc


def preprocess(x, bw1, bb1, bw2, bb2, wc1, bc1, wc2, bc2, wc3, bc3):
    """Host-side repack of full inputs into per-core input maps."""
    f32 = np.float32
    NPB = NCORES * NBR            # 1008 padded branches

    bw1p = np.zeros((NPB, H1), f32); bw1p[:N] = bw1
    bb1p = np.zeros((NPB, H1), f32); bb1p[:N] = bb1
    bw2p = np.zeros((NPB, H2, H1), f32); bw2p[:N] = bw2
    bb2p = np.zeros((NPB, H2), f32); bb2p[:N] = bb2
    xp = np.zeros((NPB, B), f32); xp[:N] = np.asarray(x).T

    wc1r = np.asarray(wc1).reshape(C1, N, H2)

    # int8 wire quantization scales (runtime-adaptive; eps guards keep
    # degenerate all-zero inputs from producing 0/0 NaNs)
    sx = max(float(np.abs(x).max()) / 127.0, 1e-20)  # global scale for x
    swc = np.maximum(np.abs(wc1).max(axis=1) / 127.0, 1e-20)  # per wc1 row
    swc_sb = np.ascontiguousarray(swc.reshape(2, 128).T.astype(f32))
    wc1_colscale = np.tile(swc, NPAIR)               # col 256t+mm -> swc[mm]

    # tail constants (replicated on all cores; small)
    wc2_sb = np.ascontiguousarray(
        wc2.T.reshape(2, 128, C2).transpose(1, 0, 2).reshape(128, 256)
    ).astype(NPBF16)
    bc1_sb = np.ascontiguousarray(bc1.reshape(2, 128).T.astype(f32))
    bc2_sb = np.ascontiguousarray(bc2.reshape(C2, 1).astype(f32))
    wc3_sb = np.ascontiguousarray(wc3.T).astype(NPBF16)
    bc3p = np.zeros(384, f32); bc3p[:OUT] = bc3
    bc3_sb = np.ascontiguousarray(bc3p.reshape(3, 128).T)

    in_maps = []
    for c in range(NCORES):
        n0 = c * NBR
        lw1 = np.zeros((NBP, H1), f32); lw1[:NBR] = bw1p[n0:n0 + NBR]
        lb1 = np.zeros((NBP, H1), f32); lb1[:NBR] = bb1p[n0:n0 + NBR]
        lw2 = np.zeros((NBP, H2, H1), f32); lw2[:NBR] = bw2p[n0:n0 + NBR]
        lb2 = np.zeros((NBP, H2), f32); lb2[:NBR] = bb2p[n0:n0 + NBR]
        lx = np.zeros((NBP, B), f32); lx[:NBR] = xp[n0:n0 + NBR]

        # scale/bias [128, NG]: row 16j+k, col g -> lw1[8g+j, k]
        # (x's int8 dequant scale sx is folded into sc)
        sc = lw1.reshape(NG, 8, H1).transpose(1, 2, 0).reshape(128, NG) * sx
        sbb = lb1.reshape(NG, 8, H1).transpose(1, 2, 0).reshape(128, NG)

        # W2 semi-compact [128, NG*8]: [16j+k, 8g+o] = lw2[8g+j, o, k]
        # (expanded to block-diagonal on device)
        lw2g = lw2.reshape(NG, 8, H2, H1)           # [g, j, o, k]
        w2_sb = np.ascontiguousarray(
            lw2g.transpose(1, 3, 0, 2).reshape(128, NG * 8)).astype(NPBF16)

        # b2 [128, NPAIR]: row 64h+8j+o, col t -> lb2[16t+8h+j, o]
        b2_sb = np.ascontiguousarray(
            lb2.reshape(NPAIR, 2, 8, H2).transpose(1, 2, 3, 0).reshape(128, NPAIR))

        # wc1 K-slice [128, NPAIR*C1]: row 64h+8j+o of chunk t,
        # col 256t+mm -> wc1[mm, 8*(n0+16t+8h+j)+o]; zero rows for pads
        wc1l = np.zeros((NBP, H2, C1), f32)         # [local branch, o, mm]
        gidx = n0 + np.arange(NBP)
        valid = (np.arange(NBP) < NBR) & (gidx < N)
        vi = np.where(valid)[0]
        wc1l[vi] = wc1r[:, gidx[vi], :].transpose(1, 2, 0)
        wc1f = wc1l.reshape(NPAIR, 2, 8, H2, C1) \
            .transpose(1, 2, 3, 0, 4).reshape(128, NPAIR * C1)
        wc1_sb = np.ascontiguousarray(
            np.clip(np.round(wc1f / wc1_colscale[None, :]), -127, 127)
            .astype(np.int8))

        tensors = {
            "xt": np.ascontiguousarray(
                np.clip(np.round(lx / sx), -127, 127).astype(np.int8)),
            "sc": np.ascontiguousarray(sc.astype(f32)),
            "sb": np.ascontiguousarray(sbb.astype(f32)),
            "w2": np.ascontiguousarray(w2_sb), "b2": b2_sb, "wc1": wc1_sb,
            "swc": swc_sb, "wc2": wc2_sb, "bc1": bc1_sb, "bc2": bc2_sb,
            "wc3": wc3_sb, "bc3": bc3_sb,
        }
        blob = np.concatenate(
            [np.ascontiguousarray(tensors[n]).view(np.uint8).reshape(128, -1)
             for n, _, _ in BLOB_ORDER], axis=1)
        assert blob.shape == (128, BLOB_BYTES)
        in_maps.append({"blob": np.ascontiguousarray(blob)})
    return in_maps


def run(in_maps, trace=False):
    nc = _build_program()
    return run_bass_kernel_spmd(nc, in_maps, list(range(NCORES)), trace=trace)


def kernel(x, bw1, bb1, bw2, bb2, wc1, bc1, wc2, bc2, wc3, bc3):
    args = [np.asarray(a, np.float32) for a in
            (x, bw1, bb1, bw2, bb2, wc1, bc1, wc2, bc2, wc3, bc3)]
    in_maps = preprocess(*args)
    res = run(in_maps, trace=False)
    y = np.empty((B, OUT), np.float32)
    for c in range(NCORES):
        y[BC * c:BC * (c + 1), :] = res.results[c]["out"].T.astype(np.float32)
    return y
